# revision 2
# baseline (speedup 1.0000x reference)
"""Deformable conv (B=4, C=256, H=W=64, O=256, K=3, pad=1) on 8 NeuronCores.

Instruction-count-minimized redesign (~280 instrs/core vs ~1630 baseline).

Sharding: core = (image b, row-half): out[b, :, half*32:(half+1)*32, :].

Host precomputes per (position, tap): one row index into a 4-corner-packed
image xtC[4096, 1024] (TL|TR|BL|BR x 256ch, fp16) and 4 corner weights
(bilinear x valid-mask, fp16, edge cases folded in).

Device pipeline per core (4 groups of 4608 pos-taps):
  1. GpSimd transpose-mode dma_gather: 6 calls x 768 idx -> [128ch, blk, j]
     slabs (channel-major, GEMM-ready layout; no PE transposes needed).
  2. One broadcast-DMA replicates the group's packed corner weights
     [4*4608] fp16 across 128 partitions.
  3. DVE combines 4 corners: 7 big strided ops (+ drains).
  4. PE GEMM: per group 36 matmuls ([128c,128o]^T x [128c,512p], 18 ck-tiles
     x 2 och halves), fp16 with fp32 PSUM accumulation.
"""

import numpy as np

B, C, H, W = 4, 256, 64, 64
O, KK = 256, 9
HW = H * W                      # 4096
ROWS_PER_CORE = H // 2          # 32
P_CORE = ROWS_PER_CORE * W      # 2048 positions per core
N_CH = P_CORE // 128            # 16 chunks (128 positions each)
N_J = KK * P_CORE               # 18432 pos-taps per core
CALL_IDX = 768                  # idx per gather call (multiple of 128)
N_CALL = N_J // CALL_IDX        # 24
N_G = 4                         # groups (GEMM granularity: 512 positions)
CALLS_PG = N_CALL // N_G        # 6
J_PG = N_J // N_G               # 4608
N_CKT = 2 * KK                  # 18 contraction tiles of 128
N_CORES = 8

_CACHE = {}


def _build_nc(reps=1, skip=()):
    skip = set(skip)
    import concourse.bacc as bacc
    import concourse.mybir as mybir
    from concourse import library_config
    import bass_rust

    F32, F16, I16 = mybir.dt.float32, mybir.dt.float16, mybir.dt.int16
    AF = mybir.ActivationFunctionType
    AL = mybir.AluOpType

    nc = bacc.Bacc("TRN2")

    xtc = nc.declare_dram_parameter("xtc", [HW, 4 * C], F16, isOutput=False)
    wt = nc.declare_dram_parameter("wt", [N_CKT, 128, O], F16, isOutput=False)
    wts = nc.declare_dram_parameter("wts", [N_G, 4 * J_PG], F16, isOutput=False)
    idx = nc.declare_dram_parameter("idx", [128, N_CALL * (CALL_IDX // 16)], I16,
                                    isOutput=False)
    out = nc.declare_dram_parameter("out", [O, P_CORE], F32, isOutput=True)

    from contextlib import ExitStack
    st = ExitStack()
    sb = lambda n, s, d: st.enter_context(nc.sbuf_tensor(n, s, d))
    ps_ = lambda n, s, d: st.enter_context(nc.psum_tensor(n, s, d))

    gbuf = sb("gbuf", [128, CALLS_PG, 8, CALL_IDX], F16)      # 72 KB/part
    cols = [sb(f"cols{i}", [128, 2, J_PG], F16) for i in range(2)]
    wrep = sb("wrep", [128, 4, J_PG], F16)                    # 36.9 KB/part
    tmp = sb("tmp", [128, 2, J_PG], F16)
    wt_sb = sb("wt_sb", [128, N_CKT, O], F16)
    idx_sb = sb("idx_sb", [128, N_CALL * (CALL_IDX // 16)], I16)
    out_sb = sb("out_sb", [128, 2, P_CORE], F32)
    ps = [ps_(f"ps{ob}", [128, N_G, 512], F32) for ob in range(2)]  # 4 banks each

    def ap3(base, dims):
        v = base.copy()
        v.ap = bass_rust.VecI64Pair(dims)
        return v

    def emit_body():
        s_io = nc.alloc_semaphore("s_io")
        s_w = nc.alloc_semaphore("s_w")
        s_g = nc.alloc_semaphore("s_g")
        s_comb = nc.alloc_semaphore("s_comb")
        s_mm = nc.alloc_semaphore("s_mm")
        s_oc = nc.alloc_semaphore("s_oc")
        s_st = nc.alloc_semaphore("s_st")
        blk_cm = nc.Block()
        block = blk_cm.__enter__()

        @block.sync
        def _(sync):
            sync.dma_start(wt_sb[:], wt.rearrange("t c e -> c t e")).then_inc(s_io, 16)
            sync.dma_start(idx_sb[:], idx[:]).then_inc(s_io, 16)
            for g in range(N_G):
                if g >= 1:
                    sync.wait_ge(s_comb, g)
                src = ap3(wts[g:g + 1, :], [[0, 128], [1, 4 * J_PG]])
                sync.dma_start(wrep[:], src).then_inc(s_w, 16)
            sync.wait_ge(s_oc, 2 * N_G)
            sync.dma_start(out[0:128, :], out_sb[:, 0, :]).then_inc(s_st, 16)
            sync.dma_start(out[128:256, :], out_sb[:, 1, :]).then_inc(s_st, 16)
            sync.wait_ge(s_st, 32)

        @block.gpsimd
        def _(gpsimd):
            gpsimd.load_library(library_config.mlp)
            gpsimd.wait_ge(s_io, 32)
            for g in range(N_G):
                if g >= 1:
                    gpsimd.wait_ge(s_comb, g)
                for m in range(CALLS_PG):
                    call = g * CALLS_PG + m
                    if "gather" in skip:
                        gpsimd.sem_inc(s_g, 16)
                        continue
                    gpsimd.dma_gather(
                        gbuf[:, m, :, :], xtc[:],
                        idx_sb[:, call * 48:(call + 1) * 48],
                        CALL_IDX, CALL_IDX, 4 * C,
                        transpose=True).then_inc(s_g, 16)

        @block.vector
        def _(vector):
            pstride_g = gbuf[:].ap[0][0]
            pstride_c = cols[0][:].ap[0][0]
            pstride_w = wrep[:].ap[0][0]
            for g in range(N_G):
                vector.wait_ge(s_g, 16 * CALLS_PG * (g + 1))
                vector.wait_ge(s_w, 16 * (g + 1))
                if g >= 2:
                    vector.wait_ge(s_mm, 2 * (g - 1))
                cg = cols[g % 2]
                if "combine" in skip:
                    vector.drain().then_inc(s_comb, 1)
                    continue

                def gv(cnr):
                    return gbuf[:, :, 2 * cnr:2 * cnr + 2, :]

                def wv(cnr):
                    return ap3(wrep[:, cnr, :],
                               [[pstride_w, 128], [CALL_IDX, CALLS_PG],
                                [0, 2], [1, CALL_IDX]])

                def ov(t):
                    return ap3(t[:, 0, 0:1],
                               [[pstride_c, 128], [CALL_IDX, CALLS_PG],
                                [J_PG, 2], [1, CALL_IDX]])

                vector.tensor_tensor(ov(cg), gv(0), wv(0), AL.mult)
                vector.tensor_tensor(ov(tmp), gv(1), wv(1), AL.mult)
                vector.drain()
                vector.tensor_tensor(ov(cg), ov(cg), ov(tmp), AL.add)
                vector.drain()
                vector.tensor_tensor(ov(tmp), gv(2), wv(2), AL.mult)
                vector.drain()
                vector.tensor_tensor(ov(cg), ov(cg), ov(tmp), AL.add)
                vector.drain()
                vector.tensor_tensor(ov(tmp), gv(3), wv(3), AL.mult)
                vector.drain()
                vector.tensor_tensor(ov(cg), ov(cg), ov(tmp), AL.add)
                vector.drain().then_inc(s_comb, 1)

        @block.tensor
        def _(tensor):
            tensor.wait_ge(s_io, 32)
            for g in range(N_G):
                tensor.wait_ge(s_comb, g + 1)
                cg = cols[g % 2]
                pstride_c = cg[:].ap[0][0]
                for ob in range(2):
                    if "pe" in skip:
                        tensor.sem_inc(s_mm, 1)
                        continue
                    for t in range(N_CKT):
                        k, b = t // 2, t % 2
                        base = cg[:, b, k * 128:k * 128 + 128]
                        rhs = base.copy()
                        rhs.ap = bass_rust.VecI64Pair(
                            [[pstride_c, 128], [KK * 128, N_CH // N_G], [1, 128]])
                        mm = tensor.matmul(ps[ob][:, g, :],
                                           wt_sb[:, t, ob * 128:(ob + 1) * 128],
                                           rhs, start=(t == 0), stop=(t == N_CKT - 1))
                    mm.then_inc(s_mm, 1)

        @block.scalar
        def _(scalar):
            if "act" in skip:
                scalar.sem_inc(s_oc, 2 * N_G)
            else:
                scalar.wait_ge(s_mm, 2 * N_G)
                for ob in range(2):
                    scalar.activation(out_sb[:, ob, :], ps[ob][:],
                                      AF.Copy).then_inc(s_oc, N_G)

        blk_cm.__exit__(None, None, None)

    snap = nc._state.snapshot_sems()
    for rep in range(reps):
        emit_body()
        if rep < reps - 1:
            nc.clear_and_free_semaphores(nc._state.allocated_since(snap))
            nc.all_engine_barrier()
            nc._state.restore_sems(snap)

    st.close()
    nc.compile()
    return nc


def _host_prep(x, offset, weight):
    """Build the 8 per-core input maps."""
    f16 = np.float16

    # 4-corner-packed images: xtc[y*64+x, cnr*256+ch], zero-padded at y=64/x=64
    xtcs = []
    for b in range(B):
        xp = np.zeros((H + 1, W + 1, C), dtype=np.float32)
        xp[:H, :W, :] = x[b].transpose(1, 2, 0)
        xtc = np.stack([xp[0:H, 0:W], xp[0:H, 1:W + 1],
                        xp[1:H + 1, 0:W], xp[1:H + 1, 1:W + 1]], axis=2)
        xtcs.append(np.ascontiguousarray(xtc.reshape(HW, 4 * C)).astype(f16))

    # GEMM weights: wt[t=2k+cb, c, o] = weight[o, cb*128+c, k]
    wtarr = np.ascontiguousarray(
        weight.reshape(O, 2, 128, KK).transpose(3, 1, 2, 0)
    ).reshape(N_CKT, 128, O).astype(f16)

    ky, kx = np.meshgrid(np.arange(3), np.arange(3), indexing="ij")
    ky = ky.reshape(-1).astype(np.float32)
    kx = kx.reshape(-1).astype(np.float32)

    in_maps = []
    for core in range(N_CORES):
        b, hhalf = core // 2, core % 2
        i0 = hhalf * ROWS_PER_CORE
        off = offset[b].reshape(KK, 2, H, W)[:, :, i0:i0 + ROWS_PER_CORE, :]
        offy = off[:, 0].reshape(KK, P_CORE).astype(np.float32)
        offx = off[:, 1].reshape(KK, P_CORE).astype(np.float32)
        p = np.arange(P_CORE)
        py = (i0 + p // W - 1).astype(np.float32)[None, :] + ky[:, None] + offy
        px = (p % W - 1).astype(np.float32)[None, :] + kx[:, None] + offx

        y0 = np.floor(py)
        x0 = np.floor(px)
        dy = (py - y0)
        dx = (px - x0)
        y0 = y0.astype(np.int64)
        x0 = x0.astype(np.int64)
        ry = np.clip(y0, 0, H - 1)
        rx = np.clip(x0, 0, W - 1)
        r = (ry * W + rx).astype(np.int16)          # [KK, P_CORE]

        def slot_w(v0, rr, d):
            # weight of slots j=0,1 given floor v0, clamped rr, frac d
            w = []
            for j in (0, 1):
                t = rr + j - v0                      # needed corner index
                val = np.where(t == 0, (1.0 - d) * ((v0 >= 0) & (v0 < H)),
                               np.where(t == 1, d * ((v0 + 1 >= 0) & (v0 + 1 < H)),
                                        0.0))
                w.append(val.astype(np.float32))
            return w

        wy = slot_w(y0, ry, dy)
        wx = slot_w(x0, rx, dx)
        wslot = np.stack([wy[0] * wx[0], wy[0] * wx[1],
                          wy[1] * wx[0], wy[1] * wx[1]])    # [4, KK, P_CORE]

        # J-order: J = c*1152 + k*128 + p  (arrays are [KK, 16*128])
        def to_j(a):
            return np.ascontiguousarray(
                a.reshape(-1, KK, N_CH, 128).transpose(0, 2, 1, 3)
            ).reshape(a.shape[0] if a.ndim == 3 else 1, N_J)

        r_j = to_j(r[None])[0]                       # [N_J] int16
        w_j = to_j(wslot)                            # [4, N_J]

        # idx wrapped-16 per call
        idxw = r_j.reshape(N_CALL, CALL_IDX // 16, 16).transpose(0, 2, 1)
        idxw = np.ascontiguousarray(idxw).reshape(N_CALL, 16, CALL_IDX // 16)
        idxw = np.concatenate([idxw[m] for m in range(N_CALL)], axis=1)  # [16, 24*48]
        idxw = np.tile(idxw, (8, 1))                 # [128, 1152]

        # weights packed per group: wts[g, cnr*4608 + j_local]
        wpk = np.ascontiguousarray(
            w_j.reshape(4, N_G, J_PG).transpose(1, 0, 2)
        ).reshape(N_G, 4 * J_PG).astype(f16)

        in_maps.append({
            "xtc": xtcs[b], "wt": wtarr, "wts": wpk, "idx": idxw,
        })
    return in_maps


def _assemble(results):
    out = np.empty((B, O, H, W), dtype=np.float32)
    for core in range(N_CORES):
        b, hhalf = core // 2, core % 2
        i0 = hhalf * ROWS_PER_CORE
        out[b, :, i0:i0 + ROWS_PER_CORE, :] = \
            np.asarray(results[core]["out"]).reshape(O, ROWS_PER_CORE, W)
    return out


def kernel(x, offset, weight):
    from concourse.bass_utils import run_bass_kernel_spmd
    x = np.asarray(x, dtype=np.float32)
    offset = np.asarray(offset, dtype=np.float32)
    weight = np.asarray(weight, dtype=np.float32)
    if "nc" not in _CACHE:
        _CACHE["nc"] = _build_nc()
    nc = _CACHE["nc"]
    in_maps = _host_prep(x, offset, weight)
    res = run_bass_kernel_spmd(nc, in_maps, list(range(N_CORES)))
    return _assemble(res.results)
